# revision 15
# baseline (speedup 1.0000x reference)
"""Trainium2 Bass kernel for nn_MixedHeadsV2 (mixed-head causal attention).

Full inputs in, full output out. Sharding: 8 cores = 4 batches x 2 head-groups.
Each core handles one batch and 4 of the 8 base heads: even cores heads
{0,1,4,5}, odd cores {2,3,6,7}. Heads 0-3 ("heavy") have effective head size
128; heads 4-7 ("light") have effective head size 64 (their mixed weight rows
64:128 are exactly zero), so the two light heads are packed into one 128-wide
tensor for projections and run as two concurrent K=64 row-tiled matmuls in
attention.

Host-side prep (layout only): x is pre-transposed to [C, T] and the base
weights are pre-transposed/packed to [3, C, 128] (heavy0, heavy1,
light0[0:64]|light1[0:64]), so the kernel needs no PE transposes at all.

Per-core pipeline (all on one NeuronCore, Tile-scheduled):
  1. eff^T mixing patterns are partition-constant: built with two K=4
     matmuls (lhsT = per-config weight column, rhs = 0/1 masks).
  2. W^T = base^T * eff^T on DVE (bf16 out).
  3. x^T tiles: DMA f32, cast to bf16 on GPSIMD.
  4. Projections q^T,k^T (d-major) and v (t-major, ones column fused for the
     softmax denominator).
  5. Causal attention, scoresT layout [s128, t512]: scores = k^T.T @ q^T into
     3-bank PSUM groups with diagonal tiles shrunk to the causal width,
     exp on ACT (scale folded; no max-subtraction: |scaled scores| < 3),
     causal masking of diagonal 128-blocks in-place on GPSIMD via
     affine_select, AV with fused row-sum (N=d+1), normalize on DVE.
     Light heads: score matmuls issued as (A,B) pairs on disjoint PE row
     groups (partitions 0:64 / 64:128) for 2x concurrency.
"""
import sys

for p in ("/opt/trn_rl_repo",):
    if p not in sys.path:
        sys.path.append(p)

import ml_dtypes
import numpy as np

import concourse.bass as bass
import concourse.tile as tile
from concourse import bacc, mybir
from concourse.bass_utils import run_bass_kernel_spmd

FP32 = mybir.dt.float32
BF16 = mybir.dt.bfloat16
AF = mybir.ActivationFunctionType
ALU = mybir.AluOpType

T = 2048
C = 512
HS = 128          # heavy head size (= padded head size)
NT128 = T // 128  # 16
NT512 = T // 512  # 4
NCC = C // 128    # 4
SCALE = float(1.0 / np.sqrt(128.0))
SGRP = 2          # score tiles (512 wide) per exp group; 2 PSUM banks

_CACHE = {}


def _build():
    nc = bacc.Bacc("TRN2", target_bir_lowering=False, debug=False, num_devices=8)
    xt_d = nc.dram_tensor("xt", [C, T], FP32, kind="ExternalInput")
    w_d = nc.dram_tensor("w", [4, 1], FP32, kind="ExternalInput")
    bq_d = nc.dram_tensor("bq", [3, HS, C], FP32, kind="ExternalInput")
    bk_d = nc.dram_tensor("bk", [3, HS, C], FP32, kind="ExternalInput")
    bv_d = nc.dram_tensor("bv", [HS, 3 * C], FP32, kind="ExternalInput")
    mA_d = nc.dram_tensor("mA", [4, C], BF16, kind="ExternalInput")
    mB_d = nc.dram_tensor("mB", [4, C], BF16, kind="ExternalInput")
    out_d = nc.dram_tensor("out", [T, 4 * HS], FP32, kind="ExternalOutput")

    with tile.TileContext(nc) as tc:
        _emit(nc, tc, xt_d, w_d, bq_d, bk_d, bv_d, mA_d, mB_d, out_d)
    nc.compile()
    return nc


def _emit(nc, tc, xt_d, w_d, bq_d, bk_d, bv_d, mA_d, mB_d, out_d):
    from contextlib import ExitStack

    ctx = ExitStack()
    prep_ctx = ExitStack()
    with ctx:
        # ---- persistent SBUF pools ----
        const_p = ctx.enter_context(tc.tile_pool(name="const", bufs=1))
        wts_p = ctx.enter_context(tc.tile_pool(name="wts", bufs=1))
        xt_p = ctx.enter_context(tc.tile_pool(name="xt", bufs=1))
        qk_p = ctx.enter_context(tc.tile_pool(name="qk", bufs=1))
        v_p = ctx.enter_context(tc.tile_pool(name="v", bufs=1))
        pt_p = ctx.enter_context(tc.tile_pool(name="pt", bufs=1))
        o_p = ctx.enter_context(tc.tile_pool(name="o", bufs=6))
        r_p = ctx.enter_context(tc.tile_pool(name="r", bufs=6))
        # ---- PSUM pools: 2-bank score groups x3 + 1-bank small x2 = 8 banks
        sps = ctx.enter_context(tc.tile_pool(name="sps", bufs=3, space="PSUM"))
        ps = ctx.enter_context(tc.tile_pool(name="ps", bufs=2, space="PSUM"))
        xstage_p = ctx.enter_context(tc.tile_pool(name="xstage", bufs=4))
        stage_p = prep_ctx.enter_context(tc.tile_pool(name="stage", bufs=3))

        # ================= eff^T patterns (partition-constant) ===========
        # effAT[p, cc*128+d] = sum_i w_i * mA[i, cc*128+d]; mA/mB are shipped
        # architecture constants (the padded-slice masks), so eff is two K=4
        # matmuls with lhsT = w broadcast along the partition-constant M dim.
        w4 = const_p.tile([4, 1], FP32, tag="w4")
        nc.sync.dma_start(w4[:], w_d.ap())
        mA = const_p.tile([4, C], BF16, tag="mA")
        nc.sync.dma_start(mA[:], mA_d.ap())
        mB = const_p.tile([4, C], BF16, tag="mB")
        nc.sync.dma_start(mB[:], mB_d.ap())
        ones4 = const_p.tile([4, 128], BF16, tag="ones4")
        nc.vector.memset(ones4[:], 1.0)
        wcol = const_p.tile([4, 128], BF16, tag="wcol")
        nc.vector.tensor_scalar_mul(wcol[:], ones4[:], w4[:])
        effAT = const_p.tile([128, C], BF16, tag="effAT")
        effBT = const_p.tile([128, C], BF16, tag="effBT")
        for eff, mask in ((effAT, mA), (effBT, mB)):
            p = ps.tile([128, 512], FP32, tag="ps")
            nc.tensor.matmul(p[:], wcol[:], mask[:], start=True, stop=True)
            nc.vector.tensor_copy(eff[:], p[:])

        # ================= effective weights (pre-transposed) ============
        # wtt[j] [128, 4*128] bf16: wtt[j][p, cc*128+d] = W_j[d, 128cc+p];
        # j in 0..8 (q h0,h1,light | k ... | v ...)
        wtt = [wts_p.tile([128, 512], BF16, name=f"wtt{j}", tag=f"wtt{j}")
               for j in range(6)]
        wt = [[wtt[j][:, cc * 128:(cc + 1) * 128] for cc in range(NCC)]
              for j in range(6)]

        def emit_wprep(hj):
            # q and k weights for head-group hj (v handled packed separately)
            for pi, bd in enumerate((bq_d, bk_d)):
                j = pi * 3 + hj
                base = stage_p.tile([128, C], FP32, tag="base", bufs=9)
                nc.scalar.dma_start(base[:], bd.ap()[hj])
                eff = effAT if hj < 2 else effBT
                nc.vector.tensor_mul(wtt[j][:], base[:], eff[:])

        # packed v weights: wttv[p, cc*384 + h*128 + d] = Wv_h[d, 128cc+p],
        # so one N=384 rhs yields all three head-groups' v per t-tile.
        wttv = wts_p.tile([128, 3 * C], BF16, tag="wttv")

        def emit_wprep_v():
            bv_all = stage_p.tile([128, 3 * C], FP32, tag="bv_all", bufs=1)
            nc.scalar.dma_start(bv_all[:], bv_d.ap())
            for cc in range(NCC):
                for hj in range(3):
                    eff = effAT if hj < 2 else effBT
                    nc.vector.tensor_mul(
                        wttv[:, cc * 384 + hj * 128:cc * 384 + (hj + 1) * 128],
                        bv_all[:, cc * 384 + hj * 128:cc * 384 + (hj + 1) * 128],
                        eff[:, cc * 128:(cc + 1) * 128])

        # ================= x^T load + cast =================
        xt_all = xt_p.tile([128, NCC * T], BF16, tag="xt_all")
        xt = [xt_all[:, cc * T:(cc + 1) * T] for cc in range(NCC)]

        def emit_x_tiles(tj):
            for cc in range(NCC):
                xs = xstage_p.tile([128, 512], FP32, name="xs", tag="xs", bufs=4)
                nc.sync.dma_start(
                    xs[:], xt_d.ap()[cc * 128:(cc + 1) * 128,
                                     tj * 512:(tj + 1) * 512])
                nc.vector.tensor_copy(
                    xt[cc][:, tj * 512:(tj + 1) * 512], xs[:])

        # ========== projections ==========
        qt = [qk_p.tile([128, T], BF16, name=f"qt{h}", tag=f"qt{h}") for h in range(3)]
        # kt: heavy0, heavy1, light-packed (rows 0:64 = lA, rows 64:128 = lB).
        kt = [qk_p.tile([128, T], BF16, name=f"kt{h}", tag=f"kt{h}") for h in range(3)]
        # unified v tile per s-tile i: [h0 v |1| h1 v |1| lA v |1| lB v |1]
        vtiles = [v_p.tile([128, 388], BF16, name=f"v_{i}", tag=f"v_{i}")
                  for i in range(NT128)]
        for i in range(NT128):
            for col in (128, 257, 322, 387):
                nc.gpsimd.memset(vtiles[i][:, col:col + 1], 1.0)

        def emit_qk_proj(hj, tj):
            for dst, j0 in ((qt, 0), (kt, 3)):
                p = ps.tile([128, 512], FP32, name="p", tag="ps")
                for cc in range(NCC):
                    nc.tensor.matmul(
                        p[:], wt[j0 + hj][cc],
                        xt[cc][:, tj * 512:(tj + 1) * 512],
                        start=(cc == 0), stop=(cc == NCC - 1))
                nc.vector.tensor_copy(dst[hj][:, tj * 512:(tj + 1) * 512], p[:])

        def emit_v_proj(i):
            # all three head-groups' v for t-tile i in one N=384 chain
            p = ps.tile([128, 512], FP32, name="p", tag="ps")
            for cc in range(NCC):
                nc.tensor.matmul(
                    p[:, 0:384], xt[cc][:, i * 128:(i + 1) * 128],
                    wttv[:, cc * 384:(cc + 1) * 384],
                    start=(cc == 0), stop=(cc == NCC - 1))
            vt_i = vtiles[i]
            nc.vector.tensor_copy(vt_i[:, 0:128], p[:, 0:128])
            nc.vector.tensor_copy(vt_i[:, 129:257], p[:, 128:256])
            nc.vector.tensor_copy(vt_i[:, 258:322], p[:, 256:320])
            nc.vector.tensor_copy(vt_i[:, 323:387], p[:, 320:384])

        def emit_av(ptile, pstride, poff, v_lo, v_hi, ocol, tj):
            # AV + fused row-sum, normalize, store. ptile slice for s-tile i,
            # t-block m: ptile[:, (pstride*i+poff)*512 + m*128 :][0:128]
            w = v_hi - v_lo
            for m in range(4):
                ti = 4 * tj + m
                op = ps.tile([128, 512], FP32, name="op", tag="ps")
                for i in range(ti + 1):
                    base = (pstride * i + poff) * 512 + m * 128
                    nc.tensor.matmul(
                        op[:, 0:w],
                        ptile[:, base:base + 128],
                        vtiles[i][:, v_lo:v_hi],
                        start=(i == 0), stop=(i == ti))
                rec = r_p.tile([128, 1], FP32, name="rec", tag="rec")
                nc.vector.reciprocal(rec[:], op[:, w - 1:w])
                ob = o_p.tile([128, 128], FP32, name="ob", tag="ob")
                nc.vector.tensor_scalar_mul(
                    ob[:, 0:w - 1], op[:, 0:w - 1], rec[:])
                nc.sync.dma_start(
                    out_d.ap()[ti * 128:(ti + 1) * 128,
                               ocol:ocol + (w - 1)],
                    ob[:, 0:w - 1])

        def emit_mask(ptile, col):
            # causal triangle on one diagonal 128x128 block, in place
            blk = ptile[:, col:col + 128]
            nc.gpsimd.affine_select(
                blk, blk, pattern=[[1, 128]],
                compare_op=ALU.is_ge, fill=0.0, base=0,
                channel_multiplier=-1)

        def emit_scores_heavy(u, tj):
            S = 4 * tj + 4
            ptile = pt_p.tile([128, NT128 * 512], BF16,
                              name=f"pth{u}_{tj}", tag="pth", bufs=2)
            for g in range(0, 4 * tj, SGRP):
                sp = sps.tile([128, SGRP * 512], FP32, name="sp", tag="sps")
                for k in range(SGRP):
                    i = g + k
                    nc.tensor.matmul(
                        sp[:, k * 512:(k + 1) * 512],
                        kt[u][:, i * 128:(i + 1) * 128],
                        qt[u][:, tj * 512:(tj + 1) * 512],
                        start=True, stop=True)
                nc.scalar.activation(
                    ptile[:, g * 512:(g + SGRP) * 512],
                    sp[:, 0:SGRP * 512], AF.Exp, scale=SCALE)
            for r in range(4):
                i = 4 * tj + r
                off = 128 * r
                sp = sps.tile([128, SGRP * 512], FP32, name="sp", tag="sps")
                nc.tensor.matmul(
                    sp[:, off:512],
                    kt[u][:, i * 128:(i + 1) * 128],
                    qt[u][:, tj * 512 + off:(tj + 1) * 512],
                    start=True, stop=True)
                nc.scalar.activation(
                    ptile[:, i * 512 + off:(i + 1) * 512],
                    sp[:, off:512], AF.Exp, scale=SCALE)
            for r in range(4):
                i = 4 * tj + r
                emit_mask(ptile, i * 512 + r * 128)
            return ptile

        ptl = pt_p.tile([128, 2 * NT128 * 512], BF16, name="ptl", tag="ptl")

        def emit_scores_light(tj):
            # both light heads; slices interleaved [A_i | B_i] in ptl; each
            # exp group holds one (A,B) pair so the K=64 row-tiled matmuls
            # stay adjacent and run concurrently on disjoint PE row groups.
            S = 4 * tj + 4
            for i in range(S):
                off = 128 * (i - 4 * tj) if i >= 4 * tj else 0
                sp = sps.tile([128, SGRP * 512], FP32, name="sp", tag="sps")
                for ab in range(2):
                    rows = slice(0, 64) if ab == 0 else slice(64, 128)
                    nc.tensor.matmul(
                        sp[:, ab * 512 + off:(ab + 1) * 512],
                        kt[2][rows, i * 128:(i + 1) * 128],
                        qt[2][rows, tj * 512 + off:(tj + 1) * 512],
                        start=True, stop=True)
                if off == 0:
                    nc.scalar.activation(
                        ptl[:, 2 * i * 512:(2 * i + 2) * 512],
                        sp[:, 0:1024], AF.Exp, scale=SCALE)
                else:
                    nc.scalar.activation(
                        ptl[:, 2 * i * 512:(2 * i + 2) * 512].rearrange(
                            "p (k t) -> p k t", k=2)[:, :, off:512],
                        sp[:, 0:1024].rearrange(
                            "p (k t) -> p k t", k=2)[:, :, off:512],
                        AF.Exp, scale=SCALE)
            for r in range(4):
                i = 4 * tj + r
                for ab in range(2):
                    emit_mask(ptl, (2 * i + ab) * 512 + r * 128)

        # ================= schedule =================
        emit_x_tiles(0)
        emit_wprep(0)
        emit_wprep(1)
        emit_wprep(2)
        emit_wprep_v()
        prep_ctx.close()
        emit_x_tiles(1)
        for tj in range(NT512):
            for hj in range(3):
                emit_qk_proj(hj, tj)
            pts = [emit_scores_heavy(0, tj), emit_scores_heavy(1, tj)]
            emit_scores_light(tj)
            for i in range(4 * tj, 4 * tj + 4):
                emit_v_proj(i)
            emit_av(pts[0], 1, 0, 0, 129, 0, tj)
            emit_av(pts[1], 1, 0, 129, 258, 128, tj)
            emit_av(ptl, 2, 0, 258, 323, 256, tj)
            emit_av(ptl, 2, 1, 323, 388, 384, tj)
            if tj + 2 < NT512:
                emit_x_tiles(tj + 2)


def _cfg_masks():
    HSL = (64, 32, 128, 64)
    EMB = (256, 256, 512, 512)
    mA = np.zeros((4, C), np.float32)
    mB = np.zeros((4, C), np.float32)
    for i in range(4):
        for cc in range(EMB[i] // 128):
            mA[i, cc * 128:cc * 128 + HSL[i]] = 1.0
    for i in (1, 3):
        hs = min(HSL[i], 64)
        for cc in range(EMB[i] // 128):
            mB[i, cc * 128:cc * 128 + hs] = 1.0
            mB[i, cc * 128 + 64:cc * 128 + 64 + hs] = 1.0
    return (mA.astype(ml_dtypes.bfloat16), mB.astype(ml_dtypes.bfloat16))


_MA, _MB = _cfg_masks()


def _shard_inputs(x, weights, base_K, base_Q, base_V):
    in_maps = []
    for c in range(8):
        b = c // 2
        hsel = [0, 1, 4, 5] if c % 2 == 0 else [2, 3, 6, 7]

        def packT(base):
            # [3, 128, 512]: bT[j][p, cc*128+d] = W_j[d, 128*cc+p]
            h0, h1, l0, l1 = (base[h] for h in hsel)
            packed = np.concatenate([l0[0:64], l1[0:64]], axis=0)  # [128, 512]
            out = np.stack([h0.T, h1.T, packed.T])  # [3, 512, 128]
            return np.ascontiguousarray(
                out.reshape(3, NCC, 128, HS).transpose(0, 2, 1, 3)
                .reshape(3, 128, C))

        def packTv(base):
            # [128, 1536]: bv[p, cc*384 + j*128 + d] = Wv_j[d, 128*cc+p]
            bT = packT(base).reshape(3, 128, NCC, HS)  # [j, p, cc, d]
            return np.ascontiguousarray(
                bT.transpose(1, 2, 0, 3).reshape(128, 3 * C))

        in_maps.append({
            "xt": np.ascontiguousarray(x[b].T),
            "w": np.ascontiguousarray(weights.reshape(4, 1)),
            "bq": packT(base_Q),
            "bk": packT(base_K),
            "bv": packTv(base_V),
            "mA": _MA,
            "mB": _MB,
        })
    return in_maps


def _gather(results):
    out = np.zeros((4, T, 8 * HS), np.float32)
    for c in range(8):
        o = results[c]["out"]
        hsel = [0, 1, 4, 5] if c % 2 == 0 else [2, 3, 6, 7]
        for j, h in enumerate(hsel):
            out[c // 2][:, h * HS:(h + 1) * HS] = o[:, j * HS:(j + 1) * HS]
    return out


def get_nc():
    if "nc" not in _CACHE:
        _CACHE["nc"] = _build()
    return _CACHE["nc"]


def kernel(x, weights, base_K, base_Q, base_V):
    x = np.asarray(x, np.float32)
    weights = np.asarray(weights, np.float32)
    base_K = np.asarray(base_K, np.float32)
    base_Q = np.asarray(base_Q, np.float32)
    base_V = np.asarray(base_V, np.float32)
    nc = get_nc()
    in_maps = _shard_inputs(x, weights, base_K, base_Q, base_V)
    res = run_bass_kernel_spmd(nc, in_maps, core_ids=list(range(8)))
    return _gather(res.results)
